# revision 39
# baseline (speedup 1.0000x reference)
"""Expert-parallel MoE SwiGLU kernel for 8 Trainium2 NeuronCores.

Problem: N=4096 tokens, top-2 of E=8 experts, H=2048, I=1408, fp32.

Strategy (load-balanced expert parallel):
  - Host-side dispatch: sort (token, k) pairs by expert. Expert counts are
    imbalanced (947..1129 here), so instead of one-expert-per-core (capacity
    = max count, padded), every core runs a UNIFORM 2-segment program:
      seg0: S0 tokens of one expert   (S0 = min feasible, 947 here)
      seg1: up to 128 tokens of one (possibly different) expert
    A small host-side solver assigns each expert's token block to seg0 of
    its home core plus <=128-token spill fragments into seg1 slots, so the
    per-core capacity is C = S0+128 = 1075 instead of 1152. 128 is the
    natural fragment floor: a fragment of any size streams the full weight
    set through the PE (LDWEIGHTS-bound below ~128 moving), so smaller
    fragments would not reduce PE time.
  - Device: per core,
        y^T = Wd_s @ (silu(Wg_s @ x^T) * (Wu_s @ x^T))   per segment s
    in [feature, token] layout, weights stationary (lhsT), tokens moving.
    Matmuls in bf16 (single-pass PE rate; abs-max rel err ~4e-3 vs fp32).
    Gate/up interleaved per h-chunk so the front x/weight DMA stream is
    consumed in arrival order.
  - Host-side combine: weighted scatter-add back to [N, H].

Weights ride as per-i "quad" packs [wg0|wg1|wu0|wu1] (one DMA per i) and
per-h-pair down packs [wd0|wd1], to keep DMA descriptor-issue cost (~0.6us
per DMA on the issuing queue) off the critical path. The i=0 quad and x are
finely sliced in consumption order to start the PE as early as possible.
"""

import numpy as np

import concourse.bass as bass
import concourse.tile as tile
from concourse import bacc, mybir
from concourse import bass_utils

N, K, E, H, I = 4096, 2, 8, 2048, 1408
P = 128
HCH = H // P   # 16 chunks over hidden dim
ICH = I // P   # 11 chunks over intermediate dim
S1 = P         # seg1 (fragment) slot width
F32 = mybir.dt.float32
BF16 = mybir.dt.bfloat16


def _plan(counts):
    """Assign expert token blocks to cores as (seg0, seg1) with minimal
    uniform S0.  Returns (S0, plan) where plan[k] = [(expert, lo, size),
    (expert, lo, size) | None] giving, per core, the slice [lo, lo+size)
    of each expert's sorted token list placed in seg0 / seg1."""
    counts = [int(c) for c in counts]
    total = sum(counts)
    lo = max(0, -(-total // E) - S1)
    for S0 in range(lo, max(max(counts), lo) + 1):
        seg0 = [None] * E   # core k hosts expert k's main block
        seg1 = [None] * E
        queue = []          # foreign fragments: (size, expert, lo)
        ok = True
        for e, c in enumerate(counts):
            m = min(c, S0)
            seg0[e] = (e, 0, m)
            rem = c - m
            if rem > 0:
                own = min(rem, S1)
                seg1[e] = (e, m, own)
                rem -= own
                if rem > S1:
                    ok = False
                    break
                if rem > 0:
                    queue.append((rem, e, m + own))
        if not ok:
            continue
        queue.sort(reverse=True)
        free = [k for k in range(E) if seg1[k] is None]
        if len(queue) > len(free):
            continue
        for (sz, e, off), k in zip(queue, free):
            seg1[k] = (e, off, sz)
        return S0, [[seg0[k], seg1[k]] for k in range(E)]
    raise AssertionError("no feasible plan")


def _chunks(S0):
    """Split seg0 into free-dim chunks <= 512 (PSUM bank limit), then the
    seg1 chunk.  Returns [(off, width, slot), ...]."""
    n = -(-S0 // 512)
    base, rem = divmod(S0, n)
    out, off = [], 0
    for j in range(n):
        w = base + (1 if j < rem else 0)
        out.append((off, w, 0))
        off += w
    out.append((S0, S1, 1))
    return out


def _build(S0):
    """Build + compile the uniform per-core 2-segment SwiGLU kernel."""
    C = S0 + S1
    ch = _chunks(S0)
    nc = bacc.Bacc("TRN2", target_bir_lowering=False, debug=False,
                   enable_asserts=False, num_devices=E)

    xT = nc.dram_tensor("xT", [H, C], BF16, kind="ExternalInput")
    # wq[i, p, (q*HCH + h)*128 + j] = W_q[i*128+j, h*128+p], q in
    # {wg seg0, wg seg1, wu seg0, wu seg1}: per-i quad, contiguous 16KB rows.
    wq = nc.dram_tensor("wq", [ICH, P, 4 * H], BF16, kind="ExternalInput")
    # wd2[h, p, (s*ICH + i)*128 + j] = Wd_s[h*128+j, i*128+p]
    wd2 = nc.dram_tensor("wd2", [HCH, P, 2 * I], BF16, kind="ExternalInput")
    outT = nc.dram_tensor("outT", [H, C], BF16, kind="ExternalOutput")

    x_r = xT.ap().rearrange("(ho p) c -> p ho c", p=P)      # [128, 16, C]
    wq_r = wq.ap()                                          # [ICH, 128, 4H]
    wq_q = wq.ap().rearrange("i p (q h) -> i p q h", q=4)   # [ICH, 128, 4, H]
    wd_t = wd2.ap().rearrange("(t two) p c -> t two p c", two=2)
    out_r = outT.ap().rearrange("(ho p) c -> p ho c", p=P)  # [128, 16, C]

    ch_e = list(enumerate(ch))
    main_ch = ch_e[:-1]
    seg1_ch = ch_e[-1:]

    with tile.TileContext(nc) as tc:
        with (
            tc.tile_pool(name="xpool", bufs=1) as xpool,
            tc.tile_pool(name="hpool", bufs=1) as hpool,
            tc.tile_pool(name="wpool", bufs=4) as wpool,
            tc.tile_pool(name="dpool", bufs=2) as dpool,
            tc.tile_pool(name="opool", bufs=2) as opool,
        ):
            from concourse.tile import add_dep_helper
            x_sb = xpool.tile([P, HCH, C], BF16)
            # hid is split so phase 2's early matmuls do not serialize behind
            # late producers (dependencies are tile-granular): hid_a for
            # i<10 mains, hid_b for i=10, hid_cu for the end-of-phase-1
            # catch-up (seg1 chunk of i=0/1)
            hid_a = hpool.tile([P, ICH - 1, C], BF16)
            hid_b = hpool.tile([P, 1, C], BF16)

            def hid(i):
                return hid_a[:, i, :] if i < ICH - 1 else hid_b[:, 0, :]

            def h_ap(i, n, c0, cw):
                return hid(i)[:, c0:c0 + cw]

            # ---- front choreography.  i=0/1 run their seg0 chunks only;
            # their seg1 chunk runs in a catch-up block at the END of phase 1
            # (slot1 weights then stay out of the front stream entirely).
            # x alternates between the two HWDGE rings (per-queue rate is
            # only ~150-230GB/s); the first x piece is split at the chunk
            # boundary so the very first matmul starts as early as possible.
            # i=0/1 seg0 gate/up slices lead the SWDGE ring in consumption
            # order, ahead of the steady quad stream.
            w_tiles = {}
            for i in (0, 1):
                w_tiles[i] = wpool.tile([P, 4, H], BF16, tag="wq",
                                        name=f"w_sb_{i}")
            x_dma = []
            x_dma.append(nc.scalar.dma_start(x_sb[:, 0:1, 0:ch[0][1]],
                                             x_r[:, 0:1, 0:ch[0][1]]))
            nc.scalar.dma_start(x_sb[:, 0:1, ch[0][1]:],
                                x_r[:, 0:1, ch[0][1]:])
            for h in range(1, HCH):
                ring = nc.scalar if h % 2 == 0 else nc.sync
                x_dma.append(ring.dma_start(x_sb[:, h:h + 1, :],
                                            x_r[:, h:h + 1, :]))
            for a, b in ((0, 2), (2, 4), (4, 8), (8, 16)):
                for i in (0, 1):
                    for q in (0, 2):
                        nc.gpsimd.dma_start(w_tiles[i][:, q, a * P:b * P],
                                            wq_q[i][:, q, a * P:b * P])
            for i in (0, 1):
                for q in (1, 3):   # slot1 halves, needed only by catch-up
                    nc.gpsimd.dma_start(w_tiles[i][:, q, :], wq_q[i][:, q, :])

            # ---- phase 1: gate/up projections + SwiGLU -> hidden^T [I, C]
            # one PSUM pool for BOTH phases: closing a pool between the
            # phases inserts an all-consumers barrier (~1.4us PE stall);
            # instead phase 2 reuses the phase-1 tags directly.
            with tc.tile_pool(name="ps", bufs=1, space="PSUM") as ps1:
                # 8 uniform-width psum tags (1 bank each, exactly 8
                # banks): T0-T3 = i-even/main family, T4-T7 = i-odd family
                # in the joint (i0,i1) block; later iterations and phase 2
                # rotate through subsets of the same tags with sliced use.
                W0 = ch[0][1]
                GT = {0: "T0", 1: "T1", 2: "T4"}
                UT = {0: "T2", 1: "T3", 2: "T6"}

                def gu_block(i, ch_i, a_order, wap, suffix=""):
                    ps_g = {n: ps1.tile([P, W0], F32,
                                        name=f"psg_{i}_{n}{suffix}",
                                        tag=GT[n])
                            for n, (c0, cw, s) in ch_i}
                    ps_u = {n: ps1.tile([P, W0], F32,
                                        name=f"psu_{i}_{n}{suffix}",
                                        tag=UT[n])
                            for n, (c0, cw, s) in ch_i}
                    loop = ([(m, h) for m in (0, 1) for h in range(HCH)]
                            if a_order else
                            [(m, h) for h in range(HCH) for m in (0, 1)])
                    for m, h in loop:
                        ps = ps_g if m == 0 else ps_u
                        for n, (c0, cw, s) in ch_i:
                            nc.tensor.matmul(
                                ps[n][:, 0:cw],
                                wap(m, h, s),
                                x_sb[:, h, c0:c0 + cw],
                                start=(h == 0),
                                stop=(h == HCH - 1),
                            )
                    for n, (c0, cw, s) in ch_i:
                        hs = h_ap(i, n, c0, cw)
                        nc.scalar.activation(
                            out=hs, in_=ps_g[n][:, 0:cw],
                            func=mybir.ActivationFunctionType.Silu,
                        )
                        nc.vector.tensor_mul(out=hs, in0=hs,
                                             in1=ps_u[n][:, 0:cw])

                # ---- joint (i0, i1) main block: processing both i's
                # h-major doubles the PE time over which x streams in, so
                # the front x/weight stream keeps up (~300GB/s needed vs
                # ~480GB/s for a single-i first iteration).  Uses all 8
                # PSUM banks: 2 i x 2 proj x 2 main chunks.
                jt_g = {(0, 0): "T0", (0, 1): "T1", (1, 0): "T4", (1, 1): "T5"}
                jt_u = {(0, 0): "T2", (0, 1): "T3", (1, 0): "T6", (1, 1): "T7"}
                psj_g = {k: ps1.tile([P, W0], F32, name=f"psjg_{k[0]}_{k[1]}",
                                     tag=v) for k, v in jt_g.items()}
                psj_u = {k: ps1.tile([P, W0], F32, name=f"psju_{k[0]}_{k[1]}",
                                     tag=v) for k, v in jt_u.items()}
                for h in range(HCH):
                    for i in (0, 1):
                        for m in (0, 1):
                            psd = psj_g if m == 0 else psj_u
                            for n, (c0, cw, s) in main_ch:
                                nc.tensor.matmul(
                                    psd[(i, n)][:, 0:cw],
                                    w_tiles[i][:, m * 2, h * P:(h + 1) * P],
                                    x_sb[:, h, c0:c0 + cw],
                                    start=(h == 0),
                                    stop=(h == HCH - 1),
                                )
                for i in (0, 1):
                    for n, (c0, cw, s) in main_ch:
                        hs = h_ap(i, n, c0, cw)
                        nc.scalar.activation(
                            out=hs, in_=psj_g[(i, n)][:, 0:cw],
                            func=mybir.ActivationFunctionType.Silu,
                        )
                        nc.vector.tensor_mul(out=hs, in0=hs,
                                             in1=psj_u[(i, n)][:, 0:cw])
                # catch-up: deferred seg1 chunk of i=0,1
                for ii in (0, 1):
                    gu_block(ii, seg1_ch, a_order=False, suffix="cu",
                             wap=(lambda w: lambda m, h, s:
                                  w[:, m * 2 + s,
                                    h * P:(h + 1) * P])(w_tiles[ii]))
                for i in range(2, ICH):
                    w_tiles[i] = wpool.tile([P, 4, H], BF16, tag="wq",
                                            name=f"w_sb_{i}")
                    nc.gpsimd.dma_start(w_tiles[i][:], wq_q[i])
                    w_sb = w_tiles[i]
                    # i=ICH-1 runs gate-then-up so the gate PSUM banks free
                    # early and phase 2 can reuse them without a stall
                    gu_block(i, ch_e, a_order=(i == ICH - 1),
                             wap=(lambda w: lambda m, h, s:
                                  w[:, m * 2 + s, h * P:(h + 1) * P])(w_sb))


                # ---- phase 2: down projection -> out^T [H, C]
                for t in range(HCH // 2):
                    wd_sb = dpool.tile([P, 2, 2 * I], BF16, tag="wd")
                    # t<4 ride the sync ring (idle mid-phase-1) so they are
                    # not queued behind the 18MB of gate/up quads on SWDGE
                    ring = nc.sync if t < 4 else nc.gpsimd
                    dd = ring.dma_start(
                        wd_sb[:], wd_t[t].rearrange("two p c -> p two c"))
                    if t < 4:
                        add_dep_helper(dd.ins, x_dma[-1].ins,
                                       reason="yield front BW to x")
                    for hh in range(2):
                        h = 2 * t + hh
                        fam = GT if h % 2 == 0 else UT
                        ps_d = [ps1.tile([P, W0], F32, name=f"psd_{h}_{n}",
                                         tag=fam[n])
                                for n, (c0, cw, s) in enumerate(ch)]
                        for i in range(ICH):
                            for n, (c0, cw, s) in enumerate(ch):
                                nc.tensor.matmul(
                                    ps_d[n][:, 0:cw],
                                    wd_sb[:, hh, s * I + i * P:s * I + (i + 1) * P],
                                    h_ap(i, n, c0, cw),
                                    start=(i == 0),
                                    stop=(i == ICH - 1),
                                )
                        o_sb = opool.tile([P, C], BF16, tag="o")
                        if h == HCH - 1:
                            # tail: copy chunks on parallel engines, write
                            # each out as it lands, smallest chunk last
                            for n, (c0, cw, s) in enumerate(ch):
                                if n % 3 == 1:
                                    nc.scalar.activation(
                                        out=o_sb[:, c0:c0 + cw],
                                        in_=ps_d[n][:, 0:cw],
                                        func=mybir.ActivationFunctionType.Copy,
                                    )
                                else:
                                    nc.vector.tensor_copy(
                                        o_sb[:, c0:c0 + cw], ps_d[n][:, 0:cw])
                            rings = [nc.sync, nc.scalar, nc.sync]
                            for n, (c0, cw, s) in enumerate(ch):
                                rings[n % 3].dma_start(
                                    out_r[:, h, c0:c0 + cw],
                                    o_sb[:, c0:c0 + cw])
                        else:
                            for n, (c0, cw, s) in enumerate(ch):
                                nc.vector.tensor_copy(o_sb[:, c0:c0 + cw],
                                                      ps_d[n][:, 0:cw])
                            nc.sync.dma_start(out_r[:, h, :], o_sb[:])

    nc.compile()
    return nc


_NC_CACHE = {}


def _get_nc(S0):
    if S0 not in _NC_CACHE:
        _NC_CACHE[S0] = _build(S0)
    return _NC_CACHE[S0]


def kernel(x, topk_ids, topk_weight, Wg, Wu, Wd):
    import ml_dtypes
    bf16 = ml_dtypes.bfloat16
    x = np.asarray(x, dtype=np.float32)
    topk_ids = np.asarray(topk_ids)
    topk_weight = np.asarray(topk_weight, dtype=np.float32)

    # ---- host-side dispatch (the all-to-all by topk_ids)
    flat = topk_ids.reshape(-1).astype(np.int64)
    order = np.argsort(flat, kind="stable")
    counts = np.bincount(flat, minlength=E)
    toks = order // K
    ks = order % K
    starts = np.cumsum(counts) - counts

    S0, plan = _plan(counts)
    C = S0 + S1
    nc = _get_nc(S0)

    def pack_gu(w):  # [I, H] -> [ICH, P, H]; out[i, p, h*128+j] = w[i*128+j, h*128+p]
        v = np.asarray(w, np.float32).reshape(ICH, P, HCH, P)
        return np.ascontiguousarray(
            v.transpose(0, 3, 2, 1).astype(bf16)).reshape(ICH, P, H)

    def pack_d(w):   # [H, I] -> [HCH, P, I]; out[h, p, i*128+j] = w[h*128+j, i*128+p]
        v = np.asarray(w, np.float32).reshape(HCH, P, ICH, P)
        return np.ascontiguousarray(
            v.transpose(0, 3, 2, 1).astype(bf16)).reshape(HCH, P, I)

    gu_cache, d_cache = {}, {}

    def gu(e):
        if e not in gu_cache:
            gu_cache[e] = (pack_gu(Wg[e]), pack_gu(Wu[e]))
        return gu_cache[e]

    def dn(e):
        if e not in d_cache:
            d_cache[e] = pack_d(Wd[e])
        return d_cache[e]

    in_maps, segs = [], []
    for k in range(E):
        xT_k = np.zeros((H, C), bf16)
        wq_k = np.zeros((ICH, P, 4, H), bf16)
        wd_k = np.zeros((HCH, P, 2, I), bf16)
        seg_k = []
        for s, seg in enumerate(plan[k]):
            if seg is None:
                seg_k.append(None)
                continue
            e, lo, sz = seg
            sl = order[starts[e] + lo: starts[e] + lo + sz]
            te, ke = toks[starts[e] + lo: starts[e] + lo + sz], ks[starts[e] + lo: starts[e] + lo + sz]
            seg_k.append((te, ke))
            col = 0 if s == 0 else S0
            xT_k[:, col:col + sz] = x[te].T.astype(bf16)
            g, u = gu(e)
            wq_k[:, :, 0 + s, :] = g
            wq_k[:, :, 2 + s, :] = u
            wd_k[:, :, s, :] = dn(e)
        segs.append(seg_k)
        in_maps.append({
            "xT": xT_k,
            "wq": np.ascontiguousarray(wq_k).reshape(ICH, P, 4 * H),
            "wd2": np.ascontiguousarray(wd_k).reshape(HCH, P, 2 * I),
        })

    res = bass_utils.run_bass_kernel_spmd(nc, in_maps, core_ids=list(range(E)))

    # ---- host-side combine (weighted scatter-add)
    out = np.zeros((N, H), np.float32)
    for k in range(E):
        yT = np.asarray(res.results[k]["outT"]).astype(np.float32)
        for s, seg in enumerate(segs[k]):
            if seg is None:
                continue
            te, ke = seg
            if len(te) == 0:
                continue
            col = 0 if s == 0 else S0
            w = topk_weight[te, ke].astype(np.float32)
            out[te] += (yT[:, col:col + len(te)] * w[None, :]).T
    return out


# revision 40
# speedup vs baseline: 1.0034x; 1.0034x over previous
"""Expert-parallel MoE SwiGLU kernel for 8 Trainium2 NeuronCores.

Problem: N=4096 tokens, top-2 of E=8 experts, H=2048, I=1408, fp32.

Strategy (load-balanced expert parallel):
  - Host-side dispatch: sort (token, k) pairs by expert. Expert counts are
    imbalanced (947..1129 here), so instead of one-expert-per-core (capacity
    = max count, padded), every core runs a UNIFORM 2-segment program:
      seg0: S0 tokens of one expert   (S0 = min feasible, 947 here)
      seg1: up to 128 tokens of one (possibly different) expert
    A small host-side solver assigns each expert's token block to seg0 of
    its home core plus <=128-token spill fragments into seg1 slots, so the
    per-core capacity is C = S0+128 = 1075 instead of 1152. 128 is the
    natural fragment floor: a fragment of any size streams the full weight
    set through the PE (LDWEIGHTS-bound below ~128 moving), so smaller
    fragments would not reduce PE time.
  - Device: per core,
        y^T = Wd_s @ (silu(Wg_s @ x^T) * (Wu_s @ x^T))   per segment s
    in [feature, token] layout, weights stationary (lhsT), tokens moving.
    Matmuls in bf16 (single-pass PE rate; abs-max rel err ~4e-3 vs fp32).
    Gate/up interleaved per h-chunk so the front x/weight DMA stream is
    consumed in arrival order.
  - Host-side combine: weighted scatter-add back to [N, H].

Weights ride as per-i "quad" packs [wg0|wg1|wu0|wu1] (one DMA per i) and
per-h-pair down packs [wd0|wd1], to keep DMA descriptor-issue cost (~0.6us
per DMA on the issuing queue) off the critical path. The i=0 quad and x are
finely sliced in consumption order to start the PE as early as possible.
"""

import numpy as np

import concourse.bass as bass
import concourse.tile as tile
from concourse import bacc, mybir
from concourse import bass_utils

N, K, E, H, I = 4096, 2, 8, 2048, 1408
P = 128
HCH = H // P   # 16 chunks over hidden dim
ICH = I // P   # 11 chunks over intermediate dim
S1 = P         # seg1 (fragment) slot width
F32 = mybir.dt.float32
BF16 = mybir.dt.bfloat16


def _plan(counts):
    """Assign expert token blocks to cores as (seg0, seg1) with minimal
    uniform S0.  Returns (S0, plan) where plan[k] = [(expert, lo, size),
    (expert, lo, size) | None] giving, per core, the slice [lo, lo+size)
    of each expert's sorted token list placed in seg0 / seg1."""
    counts = [int(c) for c in counts]
    total = sum(counts)
    lo = max(0, -(-total // E) - S1)
    for S0 in range(lo, max(max(counts), lo) + 1):
        seg0 = [None] * E   # core k hosts expert k's main block
        seg1 = [None] * E
        queue = []          # foreign fragments: (size, expert, lo)
        ok = True
        for e, c in enumerate(counts):
            m = min(c, S0)
            seg0[e] = (e, 0, m)
            rem = c - m
            if rem > 0:
                own = min(rem, S1)
                seg1[e] = (e, m, own)
                rem -= own
                if rem > S1:
                    ok = False
                    break
                if rem > 0:
                    queue.append((rem, e, m + own))
        if not ok:
            continue
        queue.sort(reverse=True)
        free = [k for k in range(E) if seg1[k] is None]
        if len(queue) > len(free):
            continue
        for (sz, e, off), k in zip(queue, free):
            seg1[k] = (e, off, sz)
        return S0, [[seg0[k], seg1[k]] for k in range(E)]
    raise AssertionError("no feasible plan")


def _chunks(S0):
    """Split seg0 into free-dim chunks <= 512 (PSUM bank limit), then the
    seg1 chunk.  Returns [(off, width, slot), ...]."""
    n = -(-S0 // 512)
    base, rem = divmod(S0, n)
    out, off = [], 0
    for j in range(n):
        w = base + (1 if j < rem else 0)
        out.append((off, w, 0))
        off += w
    out.append((S0, S1, 1))
    return out


def _build(S0):
    """Build + compile the uniform per-core 2-segment SwiGLU kernel."""
    C = S0 + S1
    ch = _chunks(S0)
    nc = bacc.Bacc("TRN2", target_bir_lowering=False, debug=False,
                   enable_asserts=False, num_devices=E)

    xT = nc.dram_tensor("xT", [H, C], BF16, kind="ExternalInput")
    # wq[i, p, (q*HCH + h)*128 + j] = W_q[i*128+j, h*128+p], q in
    # {wg seg0, wg seg1, wu seg0, wu seg1}: per-i quad, contiguous 16KB rows.
    wq = nc.dram_tensor("wq", [ICH, P, 4 * H], BF16, kind="ExternalInput")
    # wd2[h, p, (s*ICH + i)*128 + j] = Wd_s[h*128+j, i*128+p]
    wd2 = nc.dram_tensor("wd2", [HCH, P, 2 * I], BF16, kind="ExternalInput")
    outT = nc.dram_tensor("outT", [H, C], BF16, kind="ExternalOutput")

    x_r = xT.ap().rearrange("(ho p) c -> p ho c", p=P)      # [128, 16, C]
    wq_r = wq.ap()                                          # [ICH, 128, 4H]
    wq_q = wq.ap().rearrange("i p (q h) -> i p q h", q=4)   # [ICH, 128, 4, H]
    wd_t = wd2.ap().rearrange("(t two) p c -> t two p c", two=2)
    out_r = outT.ap().rearrange("(ho p) c -> p ho c", p=P)  # [128, 16, C]

    ch_e = list(enumerate(ch))
    main_ch = ch_e[:-1]
    seg1_ch = ch_e[-1:]

    with tile.TileContext(nc) as tc:
        with (
            tc.tile_pool(name="xpool", bufs=1) as xpool,
            tc.tile_pool(name="hpool", bufs=1) as hpool,
            tc.tile_pool(name="wpool", bufs=3) as wpool,
            tc.tile_pool(name="dpool", bufs=2) as dpool,
            tc.tile_pool(name="opool", bufs=2) as opool,
        ):
            from concourse.tile import add_dep_helper
            x_sb = xpool.tile([P, HCH, C], BF16)
            # hid is split so phase 2's early matmuls do not serialize behind
            # late producers (dependencies are tile-granular): hid_a for
            # i<10 mains, hid_b for i=10, hid_cu for the end-of-phase-1
            # catch-up (seg1 chunk of i=0/1)
            hid_a = hpool.tile([P, ICH - 1, C], BF16)
            hid_b = hpool.tile([P, 1, C], BF16)

            def hid(i):
                return hid_a[:, i, :] if i < ICH - 1 else hid_b[:, 0, :]

            def h_ap(i, n, c0, cw):
                return hid(i)[:, c0:c0 + cw]

            # ---- front choreography.  i=0/1 run their seg0 chunks only;
            # their seg1 chunk runs in a catch-up block at the END of phase 1
            # (slot1 weights then stay out of the front stream entirely).
            # x alternates between the two HWDGE rings (per-queue rate is
            # only ~150-230GB/s); the first x piece is split at the chunk
            # boundary so the very first matmul starts as early as possible.
            # i=0/1 seg0 gate/up slices lead the SWDGE ring in consumption
            # order, ahead of the steady quad stream.
            w_tiles = {}
            for i in (0, 1):
                w_tiles[i] = wpool.tile([P, 4, H], BF16, tag="wq",
                                        name=f"w_sb_{i}")
            x_dma = []
            x_dma.append(nc.scalar.dma_start(x_sb[:, 0:1, 0:ch[0][1]],
                                             x_r[:, 0:1, 0:ch[0][1]]))
            nc.scalar.dma_start(x_sb[:, 0:1, ch[0][1]:],
                                x_r[:, 0:1, ch[0][1]:])
            for h in range(1, HCH):
                ring = nc.scalar if h % 2 == 0 else nc.sync
                x_dma.append(ring.dma_start(x_sb[:, h:h + 1, :],
                                            x_r[:, h:h + 1, :]))
            for a, b in ((0, 2), (2, 4), (4, 16)):
                for q in (0, 2):
                    nc.gpsimd.dma_start(w_tiles[0][:, q, a * P:b * P],
                                        wq_q[0][:, q, a * P:b * P])
            for q in (0, 2):
                nc.gpsimd.dma_start(w_tiles[1][:, q, :], wq_q[1][:, q, :])
            for i in (0, 1):
                for q in (1, 3):   # slot1 halves, needed only by catch-up
                    nc.gpsimd.dma_start(w_tiles[i][:, q, :], wq_q[i][:, q, :])

            # ---- phase 1: gate/up projections + SwiGLU -> hidden^T [I, C]
            # one PSUM pool for BOTH phases: closing a pool between the
            # phases inserts an all-consumers barrier (~1.4us PE stall);
            # instead phase 2 reuses the phase-1 tags directly.
            with tc.tile_pool(name="ps", bufs=1, space="PSUM") as ps1:
                def gu_block(i, ch_i, a_order, wap, suffix=""):
                    ps_g = {n: ps1.tile([P, cw], F32,
                                        name=f"psg_{i}_{n}{suffix}",
                                        tag=f"psg{n}")
                            for n, (c0, cw, s) in ch_i}
                    ps_u = {n: ps1.tile([P, cw], F32,
                                        name=f"psu_{i}_{n}{suffix}",
                                        tag=f"psu{n}")
                            for n, (c0, cw, s) in ch_i}
                    loop = ([(m, h) for m in (0, 1) for h in range(HCH)]
                            if a_order else
                            [(m, h) for h in range(HCH) for m in (0, 1)])
                    for m, h in loop:
                        ps = ps_g if m == 0 else ps_u
                        for n, (c0, cw, s) in ch_i:
                            nc.tensor.matmul(
                                ps[n][:],
                                wap(m, h, s),
                                x_sb[:, h, c0:c0 + cw],
                                start=(h == 0),
                                stop=(h == HCH - 1),
                            )
                    for n, (c0, cw, s) in ch_i:
                        hs = h_ap(i, n, c0, cw)
                        nc.scalar.activation(
                            out=hs, in_=ps_g[n][:],
                            func=mybir.ActivationFunctionType.Silu,
                        )
                        nc.vector.tensor_mul(out=hs, in0=hs, in1=ps_u[n][:])

                for i in range(ICH):
                    if i >= 2:
                        w_tiles[i] = wpool.tile([P, 4, H], BF16, tag="wq",
                                                name=f"w_sb_{i}")
                        nc.gpsimd.dma_start(w_tiles[i][:], wq_q[i])
                    ch_i = main_ch if i < 2 else ch_e
                    w_sb = w_tiles[i]
                    # i=ICH-1 runs gate-then-up so the gate PSUM banks free
                    # early and phase 2 can reuse them without a stall
                    gu_block(i, ch_i, a_order=(i == ICH - 1),
                             wap=(lambda w: lambda m, h, s:
                                  w[:, m * 2 + s, h * P:(h + 1) * P])(w_sb))
                    if i == 1:
                        # catch-up: deferred seg1 chunk of i=0,1
                        for ii in (0, 1):
                            gu_block(ii, seg1_ch, a_order=False, suffix="cu",
                                     wap=(lambda w: lambda m, h, s:
                                          w[:, m * 2 + s,
                                            h * P:(h + 1) * P])(w_tiles[ii]))


                # ---- phase 2: down projection -> out^T [H, C]
                for t in range(HCH // 2):
                    wd_sb = dpool.tile([P, 2, 2 * I], BF16, tag="wd")
                    # t<4 ride the sync ring (idle mid-phase-1) so they are
                    # not queued behind the 18MB of gate/up quads on SWDGE
                    ring = nc.sync if t < 4 else nc.gpsimd
                    dd = ring.dma_start(
                        wd_sb[:], wd_t[t].rearrange("two p c -> p two c"))
                    if t < 4:
                        add_dep_helper(dd.ins, x_dma[-1].ins,
                                       reason="yield front BW to x")
                    for hh in range(2):
                        h = 2 * t + hh
                        fam = "psg" if h % 2 == 0 else "psu"
                        ps_d = [ps1.tile([P, cw], F32, name=f"psd_{h}_{n}",
                                         tag=f"{fam}{n}")
                                for n, (c0, cw, s) in enumerate(ch)]
                        for i in range(ICH):
                            for n, (c0, cw, s) in enumerate(ch):
                                nc.tensor.matmul(
                                    ps_d[n][:],
                                    wd_sb[:, hh, s * I + i * P:s * I + (i + 1) * P],
                                    h_ap(i, n, c0, cw),
                                    start=(i == 0),
                                    stop=(i == ICH - 1),
                                )
                        o_sb = opool.tile([P, C], BF16, tag="o")
                        if h == HCH - 1:
                            # tail: copy chunks on parallel engines, write
                            # each out as it lands, smallest chunk last
                            for n, (c0, cw, s) in enumerate(ch):
                                if n % 3 == 1:
                                    nc.scalar.activation(
                                        out=o_sb[:, c0:c0 + cw],
                                        in_=ps_d[n][:],
                                        func=mybir.ActivationFunctionType.Copy,
                                    )
                                else:
                                    nc.vector.tensor_copy(
                                        o_sb[:, c0:c0 + cw], ps_d[n][:])
                            rings = [nc.sync, nc.scalar, nc.sync]
                            for n, (c0, cw, s) in enumerate(ch):
                                rings[n % 3].dma_start(
                                    out_r[:, h, c0:c0 + cw],
                                    o_sb[:, c0:c0 + cw])
                        else:
                            for n, (c0, cw, s) in enumerate(ch):
                                nc.vector.tensor_copy(o_sb[:, c0:c0 + cw],
                                                      ps_d[n][:])
                            nc.sync.dma_start(out_r[:, h, :], o_sb[:])

    nc.compile()
    return nc


_NC_CACHE = {}


def _get_nc(S0):
    if S0 not in _NC_CACHE:
        _NC_CACHE[S0] = _build(S0)
    return _NC_CACHE[S0]


def kernel(x, topk_ids, topk_weight, Wg, Wu, Wd):
    import ml_dtypes
    bf16 = ml_dtypes.bfloat16
    x = np.asarray(x, dtype=np.float32)
    topk_ids = np.asarray(topk_ids)
    topk_weight = np.asarray(topk_weight, dtype=np.float32)

    # ---- host-side dispatch (the all-to-all by topk_ids)
    flat = topk_ids.reshape(-1).astype(np.int64)
    order = np.argsort(flat, kind="stable")
    counts = np.bincount(flat, minlength=E)
    toks = order // K
    ks = order % K
    starts = np.cumsum(counts) - counts

    S0, plan = _plan(counts)
    C = S0 + S1
    nc = _get_nc(S0)

    def pack_gu(w):  # [I, H] -> [ICH, P, H]; out[i, p, h*128+j] = w[i*128+j, h*128+p]
        v = np.asarray(w, np.float32).reshape(ICH, P, HCH, P)
        return np.ascontiguousarray(
            v.transpose(0, 3, 2, 1).astype(bf16)).reshape(ICH, P, H)

    def pack_d(w):   # [H, I] -> [HCH, P, I]; out[h, p, i*128+j] = w[h*128+j, i*128+p]
        v = np.asarray(w, np.float32).reshape(HCH, P, ICH, P)
        return np.ascontiguousarray(
            v.transpose(0, 3, 2, 1).astype(bf16)).reshape(HCH, P, I)

    gu_cache, d_cache = {}, {}

    def gu(e):
        if e not in gu_cache:
            gu_cache[e] = (pack_gu(Wg[e]), pack_gu(Wu[e]))
        return gu_cache[e]

    def dn(e):
        if e not in d_cache:
            d_cache[e] = pack_d(Wd[e])
        return d_cache[e]

    in_maps, segs = [], []
    for k in range(E):
        xT_k = np.zeros((H, C), bf16)
        wq_k = np.zeros((ICH, P, 4, H), bf16)
        wd_k = np.zeros((HCH, P, 2, I), bf16)
        seg_k = []
        for s, seg in enumerate(plan[k]):
            if seg is None:
                seg_k.append(None)
                continue
            e, lo, sz = seg
            sl = order[starts[e] + lo: starts[e] + lo + sz]
            te, ke = toks[starts[e] + lo: starts[e] + lo + sz], ks[starts[e] + lo: starts[e] + lo + sz]
            seg_k.append((te, ke))
            col = 0 if s == 0 else S0
            xT_k[:, col:col + sz] = x[te].T.astype(bf16)
            g, u = gu(e)
            wq_k[:, :, 0 + s, :] = g
            wq_k[:, :, 2 + s, :] = u
            wd_k[:, :, s, :] = dn(e)
        segs.append(seg_k)
        in_maps.append({
            "xT": xT_k,
            "wq": np.ascontiguousarray(wq_k).reshape(ICH, P, 4 * H),
            "wd2": np.ascontiguousarray(wd_k).reshape(HCH, P, 2 * I),
        })

    res = bass_utils.run_bass_kernel_spmd(nc, in_maps, core_ids=list(range(E)))

    # ---- host-side combine (weighted scatter-add)
    out = np.zeros((N, H), np.float32)
    for k in range(E):
        yT = np.asarray(res.results[k]["outT"]).astype(np.float32)
        for s, seg in enumerate(segs[k]):
            if seg is None:
                continue
            te, ke = seg
            if len(te) == 0:
                continue
            col = 0 if s == 0 else S0
            w = topk_weight[te, ke].astype(np.float32)
            out[te] += (yT[:, col:col + len(te)] * w[None, :]).T
    return out


# revision 41
# speedup vs baseline: 1.1715x; 1.1675x over previous
"""Expert-parallel MoE SwiGLU kernel for 8 Trainium2 NeuronCores.

Problem: N=4096 tokens, top-2 of E=8 experts, H=2048, I=1408, fp32.

Strategy (load-balanced expert parallel):
  - Host-side dispatch: sort (token, k) pairs by expert. Expert counts are
    imbalanced (947..1129 here), so instead of one-expert-per-core (capacity
    = max count, padded), every core runs a UNIFORM 2-segment program:
      seg0: S0 tokens of one expert   (S0 = min feasible, 947 here)
      seg1: up to 128 tokens of one (possibly different) expert
    A small host-side solver assigns each expert's token block to seg0 of
    its home core plus <=128-token spill fragments into seg1 slots, so the
    per-core capacity is C = S0+128 = 1075 instead of 1152. 128 is the
    natural fragment floor: a fragment of any size streams the full weight
    set through the PE (LDWEIGHTS-bound below ~128 moving), so smaller
    fragments would not reduce PE time.
  - Device: per core,
        y^T = Wd_s @ (silu(Wg_s @ x^T) * (Wu_s @ x^T))   per segment s
    in [feature, token] layout, weights stationary (lhsT), tokens moving.
    Matmuls in bf16 (single-pass PE rate; abs-max rel err ~4e-3 vs fp32).
    Gate/up interleaved per h-chunk so the front x/weight DMA stream is
    consumed in arrival order.
  - Host-side combine: weighted scatter-add back to [N, H].

Weights ride as per-i "quad" packs [wg0|wg1|wu0|wu1] (one DMA per i) and
per-h-pair down packs [wd0|wd1], to keep DMA descriptor-issue cost (~0.6us
per DMA on the issuing queue) off the critical path. The i=0 quad and x are
finely sliced in consumption order to start the PE as early as possible.
"""

import numpy as np

import concourse.bass as bass
import concourse.tile as tile
from concourse import bacc, mybir
from concourse import bass_utils

N, K, E, H, I = 4096, 2, 8, 2048, 1408
P = 128
HCH = H // P   # 16 chunks over hidden dim
ICH = I // P   # 11 chunks over intermediate dim
S1 = P         # seg1 (fragment) slot width
F32 = mybir.dt.float32
BF16 = mybir.dt.bfloat16


def _plan(counts):
    """Assign expert token blocks to cores as (seg0, seg1) with minimal
    uniform S0.  Returns (S0, plan) where plan[k] = [(expert, lo, size),
    (expert, lo, size) | None] giving, per core, the slice [lo, lo+size)
    of each expert's sorted token list placed in seg0 / seg1."""
    counts = [int(c) for c in counts]
    total = sum(counts)
    lo = max(0, -(-total // E) - S1)
    for S0 in range(lo, max(max(counts), lo) + 1):
        seg0 = [None] * E   # core k hosts expert k's main block
        seg1 = [None] * E
        queue = []          # foreign fragments: (size, expert, lo)
        ok = True
        for e, c in enumerate(counts):
            m = min(c, S0)
            seg0[e] = (e, 0, m)
            rem = c - m
            if rem > 0:
                own = min(rem, S1)
                seg1[e] = (e, m, own)
                rem -= own
                if rem > S1:
                    ok = False
                    break
                if rem > 0:
                    queue.append((rem, e, m + own))
        if not ok:
            continue
        queue.sort(reverse=True)
        free = [k for k in range(E) if seg1[k] is None]
        if len(queue) > len(free):
            continue
        for (sz, e, off), k in zip(queue, free):
            seg1[k] = (e, off, sz)
        return S0, [[seg0[k], seg1[k]] for k in range(E)]
    raise AssertionError("no feasible plan")


def _chunks(S0):
    """Split seg0 into free-dim chunks <= 512 (PSUM bank limit), then the
    seg1 chunk.  Returns [(off, width, slot), ...]."""
    n = -(-S0 // 512)
    base, rem = divmod(S0, n)
    out, off = [], 0
    for j in range(n):
        w = base + (1 if j < rem else 0)
        out.append((off, w, 0))
        off += w
    out.append((S0, S1, 1))
    return out


def _build(S0):
    """Build + compile the uniform per-core 2-segment SwiGLU kernel."""
    C = S0 + S1
    ch = _chunks(S0)
    nc = bacc.Bacc("TRN2", target_bir_lowering=False, debug=False,
                   enable_asserts=False, num_devices=E)

    xT = nc.dram_tensor("xT", [H, C], BF16, kind="ExternalInput")
    # wq[i, p, (q*HCH + h)*128 + j] = W_q[i*128+j, h*128+p], q in
    # {wg seg0, wg seg1, wu seg0, wu seg1}: per-i quad, contiguous 16KB rows.
    wq = nc.dram_tensor("wq", [ICH, P, 4 * H], BF16, kind="ExternalInput")
    # wd2[h, p, (s*ICH + i)*128 + j] = Wd_s[h*128+j, i*128+p]
    wd2 = nc.dram_tensor("wd2", [HCH, P, 2 * I], BF16, kind="ExternalInput")
    outT = nc.dram_tensor("outT", [H, C], BF16, kind="ExternalOutput")

    x_r = xT.ap().rearrange("(ho p) c -> p ho c", p=P)      # [128, 16, C]
    wq_r = wq.ap()                                          # [ICH, 128, 4H]
    wq_q = wq.ap().rearrange("i p (q h) -> i p q h", q=4)   # [ICH, 128, 4, H]
    wd_t = wd2.ap().rearrange("(t two) p c -> t two p c", two=2)
    out_r = outT.ap().rearrange("(ho p) c -> p ho c", p=P)  # [128, 16, C]

    ch_e = list(enumerate(ch))
    main_ch = ch_e[:-1]
    seg1_ch = ch_e[-1:]

    with tile.TileContext(nc) as tc:
        with (
            tc.tile_pool(name="xpool", bufs=1) as xpool,
            tc.tile_pool(name="hpool", bufs=1) as hpool,
            tc.tile_pool(name="wpool", bufs=3) as wpool,
            tc.tile_pool(name="dpool", bufs=2) as dpool,
            tc.tile_pool(name="opool", bufs=2) as opool,
        ):
            from concourse.tile import add_dep_helper
            x_sb = xpool.tile([P, HCH, C], BF16)
            # hid is split so phase 2's early matmuls do not serialize behind
            # late producers (dependencies are tile-granular): hid_a for
            # i<10 mains, hid_b for i=10, hid_cu for the end-of-phase-1
            # catch-up (seg1 chunk of i=0/1)
            hid_a = hpool.tile([P, ICH - 1, C], BF16)
            hid_b = hpool.tile([P, 1, C], BF16)

            def hid(i):
                return hid_a[:, i, :] if i < ICH - 1 else hid_b[:, 0, :]

            def h_ap(i, n, c0, cw):
                return hid(i)[:, c0:c0 + cw]

            # ---- front choreography.  i=0/1 run their seg0 chunks only;
            # their seg1 chunk runs in a catch-up block at the END of phase 1
            # (slot1 weights then stay out of the front stream entirely).
            # x alternates between the two HWDGE rings (per-queue rate is
            # only ~150-230GB/s); the first x piece is split at the chunk
            # boundary so the very first matmul starts as early as possible.
            # i=0/1 seg0 gate/up slices lead the SWDGE ring in consumption
            # order, ahead of the steady quad stream.
            w_tiles = {}
            for i in (0, 1):
                w_tiles[i] = wpool.tile([P, 4, H], BF16, tag="wq",
                                        name=f"w_sb_{i}")
            x_dma = []
            x_dma.append(nc.scalar.dma_start(x_sb[:, 0:1, 0:ch[0][1]],
                                             x_r[:, 0:1, 0:ch[0][1]]))
            nc.scalar.dma_start(x_sb[:, 0:1, ch[0][1]:],
                                x_r[:, 0:1, ch[0][1]:])
            for h in range(1, HCH):
                ring = nc.scalar if h % 2 == 0 else nc.sync
                x_dma.append(ring.dma_start(x_sb[:, h:h + 1, :],
                                            x_r[:, h:h + 1, :]))
            for a, b in ((0, 2), (2, 4), (4, 16)):
                for q in (0, 2):
                    nc.gpsimd.dma_start(w_tiles[0][:, q, a * P:b * P],
                                        wq_q[0][:, q, a * P:b * P])
            for q in (0, 2):
                nc.gpsimd.dma_start(w_tiles[1][:, q, :], wq_q[1][:, q, :])
            for i in (0, 1):
                for q in (1, 3):   # slot1 halves, needed only by catch-up
                    nc.gpsimd.dma_start(w_tiles[i][:, q, :], wq_q[i][:, q, :])

            # ---- phase 1: gate/up projections + SwiGLU -> hidden^T [I, C]
            # one PSUM pool for BOTH phases: closing a pool between the
            # phases inserts an all-consumers barrier (~1.4us PE stall);
            # instead phase 2 reuses the phase-1 tags directly.
            with tc.tile_pool(name="ps", bufs=1, space="PSUM") as ps1:
                def gu_block(i, ch_i, a_order, wap, suffix=""):
                    ps_g = {n: ps1.tile([P, cw], F32,
                                        name=f"psg_{i}_{n}{suffix}",
                                        tag=f"psg{n}")
                            for n, (c0, cw, s) in ch_i}
                    ps_u = {n: ps1.tile([P, cw], F32,
                                        name=f"psu_{i}_{n}{suffix}",
                                        tag=f"psu{n}")
                            for n, (c0, cw, s) in ch_i}
                    loop = ([(m, h) for m in (0, 1) for h in range(HCH)]
                            if a_order else
                            [(m, h) for h in range(HCH) for m in (0, 1)])
                    for m, h in loop:
                        ps = ps_g if m == 0 else ps_u
                        for n, (c0, cw, s) in ch_i:
                            nc.tensor.matmul(
                                ps[n][:],
                                wap(m, h, s),
                                x_sb[:, h, c0:c0 + cw],
                                start=(h == 0),
                                stop=(h == HCH - 1),
                            )
                    for n, (c0, cw, s) in ch_i:
                        hs = h_ap(i, n, c0, cw)
                        nc.scalar.activation(
                            out=hs, in_=ps_g[n][:],
                            func=mybir.ActivationFunctionType.Silu,
                        )
                        nc.vector.tensor_mul(out=hs, in0=hs, in1=ps_u[n][:])

                for i in range(ICH):
                    if i >= 2:
                        w_tiles[i] = wpool.tile([P, 4, H], BF16, tag="wq",
                                                name=f"w_sb_{i}")
                        nc.gpsimd.dma_start(w_tiles[i][:], wq_q[i])
                    ch_i = main_ch if i < 2 else ch_e
                    w_sb = w_tiles[i]
                    # i=ICH-1 runs gate-then-up so the gate PSUM banks free
                    # early and phase 2 can reuse them without a stall
                    gu_block(i, ch_i, a_order=(i == ICH - 1),
                             wap=(lambda w: lambda m, h, s:
                                  w[:, m * 2 + s, h * P:(h + 1) * P])(w_sb))
                    if i == 1:
                        # catch-up: deferred seg1 chunk of i=0,1
                        for ii in (0, 1):
                            gu_block(ii, seg1_ch, a_order=False, suffix="cu",
                                     wap=(lambda w: lambda m, h, s:
                                          w[:, m * 2 + s,
                                            h * P:(h + 1) * P])(w_tiles[ii]))


                # ---- phase 2: down projection -> out^T [H, C]
                for t in range(HCH // 2):
                    wd_sb = dpool.tile([P, 2, 2 * I], BF16, tag="wd")
                    # t<4 ride the sync ring (idle mid-phase-1) so they are
                    # not queued behind the 18MB of gate/up quads on SWDGE
                    ring = nc.sync if t < 5 else nc.gpsimd
                    dd = ring.dma_start(
                        wd_sb[:], wd_t[t].rearrange("two p c -> p two c"))
                    if t < 5:
                        add_dep_helper(dd.ins, x_dma[-1].ins,
                                       reason="yield front BW to x")
                    for hh in range(2):
                        h = 2 * t + hh
                        fam = "psg" if h % 2 == 0 else "psu"
                        ps_d = [ps1.tile([P, cw], F32, name=f"psd_{h}_{n}",
                                         tag=f"{fam}{n}")
                                for n, (c0, cw, s) in enumerate(ch)]
                        for i in range(ICH):
                            for n, (c0, cw, s) in enumerate(ch):
                                nc.tensor.matmul(
                                    ps_d[n][:],
                                    wd_sb[:, hh, s * I + i * P:s * I + (i + 1) * P],
                                    h_ap(i, n, c0, cw),
                                    start=(i == 0),
                                    stop=(i == ICH - 1),
                                )
                        o_sb = opool.tile([P, C], BF16, tag="o")
                        if h == HCH - 1:
                            # tail: copy chunks on parallel engines, write
                            # each out as it lands, smallest chunk last
                            for n, (c0, cw, s) in enumerate(ch):
                                if n % 3 == 1:
                                    nc.scalar.activation(
                                        out=o_sb[:, c0:c0 + cw],
                                        in_=ps_d[n][:],
                                        func=mybir.ActivationFunctionType.Copy,
                                    )
                                else:
                                    nc.vector.tensor_copy(
                                        o_sb[:, c0:c0 + cw], ps_d[n][:])
                            rings = [nc.sync, nc.scalar, nc.scalar]
                            for n, (c0, cw, s) in enumerate(ch):
                                rings[n % 3].dma_start(
                                    out_r[:, h, c0:c0 + cw],
                                    o_sb[:, c0:c0 + cw])
                        else:
                            for n, (c0, cw, s) in enumerate(ch):
                                nc.vector.tensor_copy(o_sb[:, c0:c0 + cw],
                                                      ps_d[n][:])
                            nc.sync.dma_start(out_r[:, h, :], o_sb[:])

    nc.compile()
    return nc


_NC_CACHE = {}


def _get_nc(S0):
    if S0 not in _NC_CACHE:
        _NC_CACHE[S0] = _build(S0)
    return _NC_CACHE[S0]


def kernel(x, topk_ids, topk_weight, Wg, Wu, Wd):
    import ml_dtypes
    bf16 = ml_dtypes.bfloat16
    x = np.asarray(x, dtype=np.float32)
    topk_ids = np.asarray(topk_ids)
    topk_weight = np.asarray(topk_weight, dtype=np.float32)

    # ---- host-side dispatch (the all-to-all by topk_ids)
    flat = topk_ids.reshape(-1).astype(np.int64)
    order = np.argsort(flat, kind="stable")
    counts = np.bincount(flat, minlength=E)
    toks = order // K
    ks = order % K
    starts = np.cumsum(counts) - counts

    S0, plan = _plan(counts)
    C = S0 + S1
    nc = _get_nc(S0)

    def pack_gu(w):  # [I, H] -> [ICH, P, H]; out[i, p, h*128+j] = w[i*128+j, h*128+p]
        v = np.asarray(w, np.float32).reshape(ICH, P, HCH, P)
        return np.ascontiguousarray(
            v.transpose(0, 3, 2, 1).astype(bf16)).reshape(ICH, P, H)

    def pack_d(w):   # [H, I] -> [HCH, P, I]; out[h, p, i*128+j] = w[h*128+j, i*128+p]
        v = np.asarray(w, np.float32).reshape(HCH, P, ICH, P)
        return np.ascontiguousarray(
            v.transpose(0, 3, 2, 1).astype(bf16)).reshape(HCH, P, I)

    gu_cache, d_cache = {}, {}

    def gu(e):
        if e not in gu_cache:
            gu_cache[e] = (pack_gu(Wg[e]), pack_gu(Wu[e]))
        return gu_cache[e]

    def dn(e):
        if e not in d_cache:
            d_cache[e] = pack_d(Wd[e])
        return d_cache[e]

    in_maps, segs = [], []
    for k in range(E):
        xT_k = np.zeros((H, C), bf16)
        wq_k = np.zeros((ICH, P, 4, H), bf16)
        wd_k = np.zeros((HCH, P, 2, I), bf16)
        seg_k = []
        for s, seg in enumerate(plan[k]):
            if seg is None:
                seg_k.append(None)
                continue
            e, lo, sz = seg
            sl = order[starts[e] + lo: starts[e] + lo + sz]
            te, ke = toks[starts[e] + lo: starts[e] + lo + sz], ks[starts[e] + lo: starts[e] + lo + sz]
            seg_k.append((te, ke))
            col = 0 if s == 0 else S0
            xT_k[:, col:col + sz] = x[te].T.astype(bf16)
            g, u = gu(e)
            wq_k[:, :, 0 + s, :] = g
            wq_k[:, :, 2 + s, :] = u
            wd_k[:, :, s, :] = dn(e)
        segs.append(seg_k)
        in_maps.append({
            "xT": xT_k,
            "wq": np.ascontiguousarray(wq_k).reshape(ICH, P, 4 * H),
            "wd2": np.ascontiguousarray(wd_k).reshape(HCH, P, 2 * I),
        })

    res = bass_utils.run_bass_kernel_spmd(nc, in_maps, core_ids=list(range(E)))

    # ---- host-side combine (weighted scatter-add)
    out = np.zeros((N, H), np.float32)
    for k in range(E):
        yT = np.asarray(res.results[k]["outT"]).astype(np.float32)
        for s, seg in enumerate(segs[k]):
            if seg is None:
                continue
            te, ke = seg
            if len(te) == 0:
                continue
            col = 0 if s == 0 else S0
            w = topk_weight[te, ke].astype(np.float32)
            out[te] += (yT[:, col:col + len(te)] * w[None, :]).T
    return out
